# revision 24
# baseline (speedup 1.0000x reference)
"""Trainium2 Bass kernel: single-head causal attention.

Problem: x[4,2048,1024] f32; q/k/v = x@W* + b* (head dim 128);
out = softmax(causal(q k^T / sqrt(128))) @ v.

Sharding: 8 cores = 4 batches x 2 causal "wedges". Within a batch, the 16
query blocks (128 rows each) are interleaved between the two cores
(h=0 takes odd global blocks, h=1 takes even) so both cores carry an
identical static schedule: slot p attends exactly L_p = 2p+2 local key
blocks. Per-core key order is a host-side permutation of the batch's key
blocks (h=0 identity, h=1 adjacent-pair swap) that puts slot p's own
(diagonal) block at local position 2p+1; the one remaining difference
between wedges (whether local position 2p is a fully-active or a fully
masked block) is carried by a mask *input*, so a single NEFF serves all
8 cores (SPMD).

Per-core pipeline (all on one NeuronCore, Tile-scheduled):
  - k^T / v^T / q^T projections as fp32r matmuls accumulating over the
    8 m-chunks of the 1024 model dim (x^T comes pre-transposed from host,
    weights come pre-chunked so their DMA is contiguous).
  - v^T tiles are PE-transposed into v-natural bf16 tiles augmented with a
    ones column.
  - scores are computed transposed (S^T[k,q]) so that softmax(P^T) tiles
    feed the P@V matmul directly as the stationary operand; softmax uses
    no max-subtraction (scores are O(1) here) and the denominator comes
    for free from the ones column of the augmented V.
"""

import numpy as np

B, T, D, DK = 4, 2048, 1024, 128
NBLK = T // 128      # 16 key blocks per core
NSLOT = 8            # q slots per core (NSLOT*128 = 1024 q rows)
NCHUNK = D // 128    # m-chunks
SCALE = 1.0 / np.sqrt(np.float32(DK))
NEG = -30000.0
JMAJOR = ()          # j-major PV drain hurt: PE stalls on each exp
X_BF16 = True        # pass x / W as bf16: halves input DMA; costs ~input rounding

_built = None


def _build():
    from contextlib import ExitStack

    import concourse.bass as bass
    import concourse.mybir as mybir
    import concourse.tile as tile
    from concourse import bacc
    from concourse.masks import make_identity

    f32 = mybir.dt.float32
    f32r = mybir.dt.float32r
    bf16 = mybir.dt.bfloat16
    Act = mybir.ActivationFunctionType

    nc = bacc.Bacc("TRN2", target_bir_lowering=False, debug=False, num_devices=8)

    xdt = bf16 if X_BF16 else f32r
    xT = nc.dram_tensor("xT", [D, T], xdt, kind="ExternalInput").ap()
    wq = nc.dram_tensor("wq", [128, NCHUNK * DK], xdt, kind="ExternalInput").ap()
    wk = nc.dram_tensor("wk", [128, NCHUNK * DK], xdt, kind="ExternalInput").ap()
    wv = nc.dram_tensor("wv", [128, NCHUNK * DK], xdt, kind="ExternalInput").ap()
    bq = nc.dram_tensor("bq", [DK, 1], f32, kind="ExternalInput").ap()
    bks = nc.dram_tensor("bks", [DK, 1], f32, kind="ExternalInput").ap()  # bk*SCALE
    bv = nc.dram_tensor("bv", [DK, 1], f32, kind="ExternalInput").ap()
    masks = nc.dram_tensor("masks", [128, 256], bf16, kind="ExternalInput").ap()
    # output = unnormalized numerator + denominator column, bf16, slot-paired:
    # o[g, row, s, :] is slot 2g+s; the softmax divide happens on the host
    o = nc.dram_tensor(
        "o", [NSLOT // 2, 128, 2, DK + 1], bf16, kind="ExternalOutput"
    ).ap()

    with tile.TileContext(nc) as tc, ExitStack() as ctx:
        const = ctx.enter_context(tc.tile_pool(name="const", bufs=1))
        sbufs = ctx.enter_context(tc.tile_pool(name="sbufs", bufs=1))
        xt_pool = ctx.enter_context(tc.tile_pool(name="xt_pool", bufs=NCHUNK))
        out_pool = ctx.enter_context(tc.tile_pool(name="out_pool", bufs=4))

        # ---- PE warmup: emitted FIRST so the PE starts ramping its clock as
        # soon as the engines come up, while the input DMA is still in flight.
        # The clock governor boosts after ~3us of gapless execution and DROPS
        # back on a >~1.5us idle, so the warmup uses many small (256-col)
        # matmuls on two alternating psum tiles: the PE stays gaplessly busy
        # until right at first-data (~12.3us) and exits within ~0.25us of it.
        WARMUP_MMS = 21
        with tc.tile_pool(name="warmps", bufs=2, space="PSUM") as warmps:
            wsrc = sbufs.tile([128, 256], bf16, tag="wsrc")
            nc.gpsimd.memset(wsrc, 0.0)
            wdst = [warmps.tile([128, 256], f32, tag="warm", name=f"warm{i}")
                    for i in range(2)]
            for i in range(WARMUP_MMS):
                nc.tensor.matmul(
                    wdst[i % 2], lhsT=wsrc[:, 0:128], rhs=wsrc,
                    start=True, stop=True,
                )
            # pull the ~1.3us exp ACT_TABLE_LOAD out of the attention phase
            wexp = sbufs.tile([128, 1], f32, tag="wexp")
            nc.scalar.activation(out=wexp, in_=wsrc[:, 0:1], func=Act.Exp, scale=1.0)

        # ---- constants (weights come host-pre-chunked: col block c = m-chunk c)
        # Small consts ride the Scalar engine's DMA queue (Q10, slow but
        # parallel); all bulk data goes through Sync's fast Q1, wk first.
        wk_sb = const.tile([128, NCHUNK * DK], xdt, tag="wk")
        nc.sync.dma_start(out=wk_sb, in_=wk)
        bq_sb = const.tile([128, 1], f32, tag="bq")
        nc.scalar.dma_start(out=bq_sb, in_=bq)
        bks_sb = const.tile([128, 1], f32, tag="bks")
        nc.scalar.dma_start(out=bks_sb, in_=bks)
        bv_sb = const.tile([128, 1], f32, tag="bv")
        nc.scalar.dma_start(out=bv_sb, in_=bv)
        mask_sb = const.tile([128, 256], bf16, tag="mask")
        nc.scalar.dma_start(out=mask_sb, in_=masks)

        # ---- x^T chunks (kept resident: q projection re-reads them).
        # xt0 is split in half so the first k/v matmuls unblock sooner during
        # the DMA pipeline's slow rampup.
        xts = []
        for c in range(NCHUNK):
            xt = xt_pool.tile([128, T], xdt, tag="xt", name=f"xt{c}")
            xts.append(xt)

        nc.sync.dma_start(out=xts[0][:, 0 : T // 2], in_=xT[0:128, 0 : T // 2])
        nc.sync.dma_start(out=xts[0][:, T // 2 : T], in_=xT[0:128, T // 2 : T])
        nc.sync.dma_start(out=xts[1][:, 0 : T // 2], in_=xT[128:256, 0 : T // 2])
        nc.sync.dma_start(out=xts[1][:, T // 2 : T], in_=xT[128:256, T // 2 : T])
        wv_sb = const.tile([128, NCHUNK * DK], xdt, tag="wv")
        nc.sync.dma_start(out=wv_sb, in_=wv)
        for c in range(2, NCHUNK):
            nc.sync.dma_start(out=xts[c], in_=xT[128 * c : 128 * (c + 1), :])

        wq_sb = const.tile([128, NCHUNK * DK], xdt, tag="wq")
        nc.sync.dma_start(out=wq_sb, in_=wq)
        ident = const.tile([128, 128], bf16, tag="ident")
        make_identity(nc, ident)
        # v in natural [k, v] layout, bf16, with a ones column appended
        v_aug = const.tile([128, NBLK, DK + 1], bf16, tag="vaug")
        nc.vector.memset(v_aug[:, :, DK : DK + 1], 1.0)

        # ---- projections ----
        kT_sb = sbufs.tile([128, T], bf16, tag="kT")       # (k^T + bk) * SCALE
        qT_sb = sbufs.tile([128, NSLOT * 128], bf16, tag="qT")  # q^T + bq
        vT_sb = sbufs.tile([128, T], bf16, tag="vT")       # v^T + bv

        # kT gets 4 psum banks, qT 2, vT 2 (accumulated in two half-passes) --
        # all three coexist, so no projection matmul ever waits on another
        # projection's psum release.
        kpool = tc.alloc_tile_pool(name="kpool", bufs=1, space="PSUM")
        qpool = tc.alloc_tile_pool(name="qpool", bufs=1, space="PSUM")
        vpool = tc.alloc_tile_pool(name="vpool", bufs=2, space="PSUM")
        if True:
            kT_ps = kpool.tile([128, T], f32, tag="kps")
            qT_ps = qpool.tile([128, NSLOT * 128], f32, tag="qps")
            # vT accumulates in four [128,512] generations over 2 psum banks;
            # separate tiles (not one [128,1024]) so each b-generation only
            # waits on its own bank's a-copy, not the whole vTa drain
            vps_a = [
                vpool.tile([128, 512], f32, tag="vps", name=f"vpsa{t}")
                for t in range(2)
            ]
            # per chunk: kT x4, vT(first half) x2, qT x2 = 8 matmuls, which
            # matches the x^T chunk DMA arrival rate. Column order t=0,1 first
            # (k then v), then t=2,3: the first half of a chunk's columns
            # arrives first for the split early-chunk DMAs.
            for c in range(NCHUNK):
                for t in range(2):
                    nc.tensor.matmul(
                        kT_ps[:, 512 * t : 512 * (t + 1)],
                        lhsT=wk_sb[:, 128 * c : 128 * (c + 1)],
                        rhs=xts[c][:, 512 * t : 512 * (t + 1)],
                        start=(c == 0),
                        stop=(c == NCHUNK - 1),
                    )
                    nc.tensor.matmul(
                        vps_a[t],
                        lhsT=wv_sb[:, 128 * c : 128 * (c + 1)],
                        rhs=xts[c][:, 512 * t : 512 * (t + 1)],
                        start=(c == 0),
                        stop=(c == NCHUNK - 1),
                    )
                for t in range(2, 4):
                    nc.tensor.matmul(
                        kT_ps[:, 512 * t : 512 * (t + 1)],
                        lhsT=wk_sb[:, 128 * c : 128 * (c + 1)],
                        rhs=xts[c][:, 512 * t : 512 * (t + 1)],
                        start=(c == 0),
                        stop=(c == NCHUNK - 1),
                    )
                x4 = xts[c].rearrange("p (b two x) -> p b two x", two=2, x=128)
                for t in range(2):
                    nc.tensor.matmul(
                        qT_ps[:, 512 * t : 512 * (t + 1)],
                        lhsT=wq_sb[:, 128 * c : 128 * (c + 1)],
                        rhs=x4[:, 4 * t : 4 * t + 4, 1, :],
                        start=(c == 0),
                        stop=(c == NCHUNK - 1),
                    )
            # copies: kT+qT on DVE (the ACT engine is kept free so the first
            # exp can issue the moment S^T_0 lands), vT halves on DVE too.
            # 512-col granularity, interleaved so the first S^T matmul (needs
            # kT cols 0:128 + qT cols 0:512) unblocks after just two copies.
            def kT_copy(t):
                sl = slice(512 * t, 512 * (t + 1))
                nc.vector.tensor_scalar(
                    out=kT_sb[:, sl], in0=kT_ps[:, sl],
                    scalar1=float(SCALE), scalar2=bks_sb,
                    op0=mybir.AluOpType.mult, op1=mybir.AluOpType.add,
                )

            def qT_copy(t):
                sl = slice(512 * t, 512 * (t + 1))
                nc.vector.tensor_scalar_add(qT_sb[:, sl], qT_ps[:, sl], bq_sb)

            kT_copy(0)
            qT_copy(0)
            qT_copy(1)
            kT_copy(1)
            kT_copy(2)
            kT_copy(3)
            for t in range(2):
                sl = slice(512 * t, 512 * (t + 1))
                nc.vector.tensor_scalar_add(vT_sb[:, sl], vps_a[t], bv_sb)

            # vT second half accumulates while the kT/qT copies drain
            vps_b = [
                vpool.tile([128, 512], f32, tag="vps", name=f"vpsb{t}")
                for t in range(2)
            ]
            for c in range(NCHUNK):
                for t in range(2):
                    nc.tensor.matmul(
                        vps_b[t],
                        lhsT=wv_sb[:, 128 * c : 128 * (c + 1)],
                        rhs=xts[c][:, 1024 + 512 * t : 1024 + 512 * (t + 1)],
                        start=(c == 0),
                        stop=(c == NCHUNK - 1),
                    )
            for t in range(2):
                nc.vector.tensor_scalar_add(
                    vT_sb[:, 1024 + 512 * t : 1024 + 512 * (t + 1)],
                    vps_b[t], bv_sb,
                )

        # ---- attention ----
        vpool.release()
        qpool.release()
        kpool.release()
        spool = ctx.enter_context(tc.tile_pool(name="spool", bufs=3, space="PSUM"))
        # one shared 5-slot pool for transpose scratch AND output accumulators:
        # transposes need slots early in the attention phase, o_ps late, so a
        # shared pool gives each phase more slack than a static 1/4 split
        opool = ctx.enter_context(tc.tile_pool(name="opool", bufs=5, space="PSUM"))
        pt_pool = ctx.enter_context(tc.tile_pool(name="pt_pool", bufs=NBLK))

        # v^T -> v natural (bf16) via PE transpose; emitted lazily inside the
        # attention loop so the PE never stalls in a transpose block waiting
        # for the vT copies (transpose for key block j lands just before its
        # S^T matmul; burst p only needs transposes <= 2p+1, which are done)
        def emit_transpose(j):
            vt_ps = opool.tile([128, DK + 1], bf16, tag="o", name=f"vt_ps{j}")
            vt_ps = vt_ps[:, 0:128]
            nc.tensor.transpose(vt_ps, vT_sb[:, 128 * j : 128 * (j + 1)], ident)
            nc.vector.tensor_copy(v_aug[:, j, 0:DK], vt_ps)

        def chunk_sizes(n):
            # pieces <=512, all >=256 when possible (fp32r full-rate needs >=256)
            out = []
            while n > 768:
                out.append(512)
                n -= 512
            if n > 512:
                out.append(n - 256)
                n = 256
            out.append(n)
            return out

        pts = [None] * NBLK

        def pv_mm(o_ps, p, jj):
            nc.tensor.matmul(
                o_ps,
                lhsT=pts[jj][:, 128 * (p - jj // 2) : 128 * (p - jj // 2) + 128],
                rhs=v_aug[:, jj, :],
                start=(jj == 0),
                stop=(jj == 2 * p + 1),
            )

        pair_tiles = {}

        def finish_slot(o_ps, p):
            # copy numerator+denominator (bf16) into the pair staging tile;
            # odd slots copy on ACT so the last two finishes overlap engines.
            # The divide happens on the host.
            g = p // 2
            if g not in pair_tiles:
                pair_tiles[g] = out_pool.tile(
                    [128, 2, DK + 1], bf16, tag="ob", name=f"ob{g}"
                )
            ot = pair_tiles[g]
            if p % 2:
                nc.scalar.copy(ot[:, 1, :], o_ps)
            else:
                nc.vector.tensor_copy(ot[:, 0, :], o_ps)
            if p % 2:
                nc.sync.dma_start(out=o[g], in_=ot)

        # process key positions 14,15 early so the final P@V bursts never
        # wait on their exp at the very end of the kernel
        ORDER = [0, 1, 2, 3, 4, 5, 6, 7, 8, 9, 14, 15, 10, 11, 12, 13]
        done = set()
        burst_done = set()
        half_done = {}          # p -> o_ps, for slots drained progressively
        pending = []            # bursts delayed one ORDER step: their pt dep
                                # (exp) finishes while the next j's S^T matmuls
                                # keep the PE busy, hiding the ACT latency
        SPLIT = {6, 7}          # late slots: drain in three phases so only
                                # blocks 10..13 remain at the kernel tail

        def pv2(o_ps, p, jj, start, stop):
            nc.tensor.matmul(
                o_ps,
                lhsT=pts[jj][:, 128 * (p - jj // 2) : 128 * (p - jj // 2) + 128],
                rhs=v_aug[:, jj, :],
                start=start,
                stop=stop,
            )

        def emit_full(p):
            o_ps = opool.tile([128, DK + 1], f32, tag="o", name=f"o_ps{p}")
            for jj in range(2 * p + 2):
                pv_mm(o_ps, p, jj)
            finish_slot(o_ps, p)

        def emit_phaseA(p):
            o_ps = opool.tile([128, DK + 1], f32, tag="o", name=f"o_ps{p}")
            half_done[p] = o_ps
            for jj in range(8):
                pv2(o_ps, p, jj, start=(jj == 0), stop=False)

        def emit_phaseB(p):
            o_ps = half_done[p]
            for jj in ([8, 9] if p == 6 else [8, 9, 14, 15]):
                pv2(o_ps, p, jj, start=False, stop=False)

        def emit_phaseC(p):
            o_ps = half_done[p]
            for jj in range(10, 14):
                pv2(o_ps, p, jj, start=False, stop=(jj == 13))
            finish_slot(o_ps, p)

        for j in ORDER:
            sj = j // 2           # first active slot for this key position
            q0 = 128 * sj
            qn = NSLOT * 128 - q0
            pt = pt_pool.tile([128, qn], bf16, tag="pt", name=f"pt{j}")
            pts[j] = pt
            off = 0
            for sz in chunk_sizes(qn):
                s_ps = spool.tile([128, 512], f32, tag="st")
                nc.tensor.matmul(
                    s_ps[:, :sz],
                    lhsT=kT_sb[:, 128 * j : 128 * (j + 1)],
                    rhs=qT_sb[:, q0 + off : q0 + off + sz],
                    start=True,
                    stop=True,
                )
                nc.scalar.activation(
                    out=pt[:, off : off + sz], in_=s_ps[:, :sz], func=Act.Exp,
                    scale=1.0,
                )
                if off == 0:
                    # mask the frontier slot multiplicatively (exp(s+m) =
                    # exp(s)*m01): even j -> maskA (wedge-dependent), odd j ->
                    # maskB (causal triangle); bf16 SBUF op, off the psum path
                    sel = j % 2
                    nc.vector.tensor_mul(
                        pt[:, 0:128],
                        pt[:, 0:128],
                        mask_sb[:, 128 * sel : 128 * (sel + 1)],
                    )
                off += sz

            # flush bursts queued on the previous ORDER step, now that this
            # j's score matmuls are in the PE stream ahead of them
            for fn in pending:
                fn()
            pending = []

            emit_transpose(j)
            done.add(j)
            phaseA_done = len(done) >= 8 and all(jj in done for jj in range(8))
            phaseB_done = all(jj in done for jj in [8, 9, 14, 15])
            for p in range(NSLOT):
                if p in burst_done:
                    continue
                if p in SPLIT:
                    if p not in half_done and phaseA_done:
                        pending.append(lambda p=p: emit_phaseA(p))
                        half_done[p] = None  # reserved; tile set in emit_phaseA
                    elif p in half_done and (p, "B") not in burst_done and \
                            phaseA_done and phaseB_done:
                        burst_done.add((p, "B"))
                        pending.append(lambda p=p: emit_phaseB(p))
                    if (p, "B") in burst_done and all(
                        jj in done for jj in range(2 * p + 2)
                    ):
                        burst_done.add(p)
                        pending.append(lambda p=p: emit_phaseC(p))
                elif all(jj in done for jj in range(2 * p + 2)):
                    burst_done.add(p)
                    pending.append(lambda p=p: emit_full(p))
        for fn in pending:
            fn()

    nc.compile()
    return nc


def get_built():
    global _built
    if _built is None:
        _built = _build()
    return _built


def _pos2glob(h):
    if h == 0:
        return list(range(NBLK))
    return [j + 1 if j % 2 == 0 else j - 1 for j in range(NBLK)]


def _xdt():
    if X_BF16:
        import ml_dtypes
        return ml_dtypes.bfloat16
    return np.float32


def _pack_w(W):
    """[D, DK] -> [128, NCHUNK*DK] with column block c holding rows 128c..128c+127."""
    return np.ascontiguousarray(
        np.asarray(W, np.float32).reshape(NCHUNK, 128, DK).transpose(1, 0, 2)
        .reshape(128, NCHUNK * DK).astype(_xdt())
    )


def make_in_map(x_b, Wq, bq, Wk, bk, Wv, bv, h, xT_pre=None):
    """Build one core's input dict. x_b: [T, D] fp32 for this core's batch.
    xT_pre: optional precomputed x_b.T already in the kernel dtype (shared by
    both wedge cores of a batch; h=0 uses it as-is, h=1 column-permutes)."""
    if xT_pre is None:
        xT_pre = np.ascontiguousarray(x_b.T.astype(_xdt()))
    if h == 0:
        xT_loc = xT_pre  # identity key order
    else:
        p2g = _pos2glob(h)
        cols = np.concatenate([np.arange(128 * g, 128 * (g + 1)) for g in p2g])
        xT_loc = np.ascontiguousarray(xT_pre[:, cols])
    import ml_dtypes
    bf = ml_dtypes.bfloat16
    maskA = (np.ones if h == 0 else np.zeros)((128, 128), bf)
    kk = np.arange(128)
    maskB = np.where(kk[:, None] <= kk[None, :], 1.0, 0.0).astype(bf)
    return {
        "xT": xT_loc,
        "wq": _pack_w(Wq),
        "wk": _pack_w(Wk),
        "wv": _pack_w(Wv),
        "bq": np.ascontiguousarray(bq.reshape(DK, 1), np.float32),
        "bks": np.ascontiguousarray((bk * SCALE).reshape(DK, 1), np.float32),
        "bv": np.ascontiguousarray(bv.reshape(DK, 1), np.float32),
        "masks": np.ascontiguousarray(np.concatenate([maskA, maskB], axis=1)),
    }


def gather_out(results):
    """results: list of 8 dicts with 'o' [4, 128, 2, DK+1] bf16 (numerator |
    denominator, slot-paired) -> full [B, T, DK] f32 (softmax divide here)."""
    out = np.zeros((B, T, DK), np.float32)
    for core in range(8):
        b, h = core // 2, core % 2
        ob = np.asarray(results[core]["o"], np.float32)
        for p in range(NSLOT):
            g = 2 * p + 1 - h
            blk = ob[p // 2, :, p % 2, :]
            out[b, 128 * g : 128 * (g + 1), :] = blk[:, :DK] / blk[:, DK:]
    return out


def kernel(x, Wq, bq, Wk, bk, Wv, bv):
    from concourse.bass_utils import run_bass_kernel_spmd

    x = np.asarray(x, np.float32)
    args = [np.asarray(a, np.float32) for a in (Wq, bq, Wk, bk, Wv, bv)]
    nc = get_built()
    # one transpose+cast per batch, shared by its two wedge cores
    xT_pres = [np.ascontiguousarray(x[b].T.astype(_xdt())) for b in range(B)]
    in_maps = [
        make_in_map(x[core // 2], args[0], args[1], args[2], args[3], args[4],
                    args[5], core % 2, xT_pre=xT_pres[core // 2])
        for core in range(8)
    ]
    res = run_bass_kernel_spmd(nc, in_maps, core_ids=list(range(8)))
    return gather_out(res.results)


if __name__ == "__main__":
    rng = np.random.default_rng(0)
    x = rng.standard_normal((B, T, D), dtype=np.float32)
    Wq = rng.standard_normal((D, DK), dtype=np.float32) * 0.03
    out = kernel(x, Wq, np.zeros(DK, np.float32), Wq, np.zeros(DK, np.float32),
                 Wq, np.zeros(DK, np.float32))
    print(out.shape)



# revision 25
# speedup vs baseline: 1.0030x; 1.0030x over previous
"""Trainium2 Bass kernel: single-head causal attention.

Problem: x[4,2048,1024] f32; q/k/v = x@W* + b* (head dim 128);
out = softmax(causal(q k^T / sqrt(128))) @ v.

Sharding: 8 cores = 4 batches x 2 causal "wedges". Within a batch, the 16
query blocks (128 rows each) are interleaved between the two cores
(h=0 takes odd global blocks, h=1 takes even) so both cores carry an
identical static schedule: slot p attends exactly L_p = 2p+2 local key
blocks. Per-core key order is a host-side permutation of the batch's key
blocks (h=0 identity, h=1 adjacent-pair swap) that puts slot p's own
(diagonal) block at local position 2p+1; the one remaining difference
between wedges (whether local position 2p is a fully-active or a fully
masked block) is carried by a mask *input*, so a single NEFF serves all
8 cores (SPMD).

Per-core pipeline (all on one NeuronCore, Tile-scheduled):
  - k^T / v^T / q^T projections as fp32r matmuls accumulating over the
    8 m-chunks of the 1024 model dim (x^T comes pre-transposed from host,
    weights come pre-chunked so their DMA is contiguous).
  - v^T tiles are PE-transposed into v-natural bf16 tiles augmented with a
    ones column.
  - scores are computed transposed (S^T[k,q]) so that softmax(P^T) tiles
    feed the P@V matmul directly as the stationary operand; softmax uses
    no max-subtraction (scores are O(1) here) and the denominator comes
    for free from the ones column of the augmented V.
"""

import numpy as np

B, T, D, DK = 4, 2048, 1024, 128
NBLK = T // 128      # 16 key blocks per core
NSLOT = 8            # q slots per core (NSLOT*128 = 1024 q rows)
NCHUNK = D // 128    # m-chunks
SCALE = 1.0 / np.sqrt(np.float32(DK))
NEG = -30000.0
JMAJOR = ()          # j-major PV drain hurt: PE stalls on each exp
X_BF16 = True        # pass x / W as bf16: halves input DMA; costs ~input rounding

_built = None


def _build():
    from contextlib import ExitStack

    import concourse.bass as bass
    import concourse.mybir as mybir
    import concourse.tile as tile
    from concourse import bacc
    from concourse.masks import make_identity

    f32 = mybir.dt.float32
    f32r = mybir.dt.float32r
    bf16 = mybir.dt.bfloat16
    Act = mybir.ActivationFunctionType

    nc = bacc.Bacc("TRN2", target_bir_lowering=False, debug=False, num_devices=8)

    xdt = bf16 if X_BF16 else f32r
    xT = nc.dram_tensor("xT", [D, T], xdt, kind="ExternalInput").ap()
    wq = nc.dram_tensor("wq", [128, NCHUNK * DK], xdt, kind="ExternalInput").ap()
    wk = nc.dram_tensor("wk", [128, NCHUNK * DK], xdt, kind="ExternalInput").ap()
    wv = nc.dram_tensor("wv", [128, NCHUNK * DK], xdt, kind="ExternalInput").ap()
    bq = nc.dram_tensor("bq", [DK, 1], f32, kind="ExternalInput").ap()
    bks = nc.dram_tensor("bks", [DK, 1], f32, kind="ExternalInput").ap()  # bk*SCALE
    bv = nc.dram_tensor("bv", [DK, 1], f32, kind="ExternalInput").ap()
    masks = nc.dram_tensor("masks", [128, 256], bf16, kind="ExternalInput").ap()
    # output = unnormalized numerator + denominator column, bf16, slot-paired:
    # o[g, row, s, :] is slot 2g+s; the softmax divide happens on the host
    o = nc.dram_tensor(
        "o", [NSLOT // 2, 128, 2, DK + 1], bf16, kind="ExternalOutput"
    ).ap()

    with tile.TileContext(nc) as tc, ExitStack() as ctx:
        const = ctx.enter_context(tc.tile_pool(name="const", bufs=1))
        sbufs = ctx.enter_context(tc.tile_pool(name="sbufs", bufs=1))
        xt_pool = ctx.enter_context(tc.tile_pool(name="xt_pool", bufs=NCHUNK))
        out_pool = ctx.enter_context(tc.tile_pool(name="out_pool", bufs=4))

        # ---- PE warmup: emitted FIRST so the PE starts ramping its clock as
        # soon as the engines come up, while the input DMA is still in flight.
        # The clock governor boosts after ~3us of gapless execution and DROPS
        # back on a >~1.5us idle, so the warmup uses many small (256-col)
        # matmuls on two alternating psum tiles: the PE stays gaplessly busy
        # until right at first-data (~12.3us) and exits within ~0.25us of it.
        WARMUP_MMS = 21
        with tc.tile_pool(name="warmps", bufs=2, space="PSUM") as warmps:
            wsrc = sbufs.tile([128, 256], bf16, tag="wsrc")
            nc.gpsimd.memset(wsrc, 0.0)
            wdst = [warmps.tile([128, 256], f32, tag="warm", name=f"warm{i}")
                    for i in range(2)]
            for i in range(WARMUP_MMS):
                nc.tensor.matmul(
                    wdst[i % 2], lhsT=wsrc[:, 0:128], rhs=wsrc,
                    start=True, stop=True,
                )
            # pull the ~1.3us exp ACT_TABLE_LOAD out of the attention phase
            wexp = sbufs.tile([128, 1], f32, tag="wexp")
            nc.scalar.activation(out=wexp, in_=wsrc[:, 0:1], func=Act.Exp, scale=1.0)

        # ---- constants (weights come host-pre-chunked: col block c = m-chunk c)
        # Small consts ride the Scalar engine's DMA queue (Q10, slow but
        # parallel); all bulk data goes through Sync's fast Q1, wk first.
        wk_sb = const.tile([128, NCHUNK * DK], xdt, tag="wk")
        nc.sync.dma_start(out=wk_sb, in_=wk)
        bq_sb = const.tile([128, 1], f32, tag="bq")
        nc.scalar.dma_start(out=bq_sb, in_=bq)
        bks_sb = const.tile([128, 1], f32, tag="bks")
        nc.scalar.dma_start(out=bks_sb, in_=bks)
        bv_sb = const.tile([128, 1], f32, tag="bv")
        nc.scalar.dma_start(out=bv_sb, in_=bv)
        mask_sb = const.tile([128, 256], bf16, tag="mask")
        nc.scalar.dma_start(out=mask_sb, in_=masks)

        # ---- x^T chunks (kept resident: q projection re-reads them).
        # xt0 is split in half so the first k/v matmuls unblock sooner during
        # the DMA pipeline's slow rampup.
        xts = []
        for c in range(NCHUNK):
            xt = xt_pool.tile([128, T], xdt, tag="xt", name=f"xt{c}")
            xts.append(xt)

        nc.sync.dma_start(out=xts[0][:, 0 : T // 2], in_=xT[0:128, 0 : T // 2])
        nc.sync.dma_start(out=xts[0][:, T // 2 : T], in_=xT[0:128, T // 2 : T])
        nc.sync.dma_start(out=xts[1][:, 0 : T // 2], in_=xT[128:256, 0 : T // 2])
        nc.sync.dma_start(out=xts[1][:, T // 2 : T], in_=xT[128:256, T // 2 : T])
        wv_sb = const.tile([128, NCHUNK * DK], xdt, tag="wv")
        nc.sync.dma_start(out=wv_sb, in_=wv)
        for c in range(2, NCHUNK):
            nc.sync.dma_start(out=xts[c], in_=xT[128 * c : 128 * (c + 1), :])

        wq_sb = const.tile([128, NCHUNK * DK], xdt, tag="wq")
        nc.sync.dma_start(out=wq_sb, in_=wq)
        ident = const.tile([128, 128], bf16, tag="ident")
        make_identity(nc, ident)
        # v in natural [k, v] layout, bf16, with a ones column appended
        v_aug = const.tile([128, NBLK, DK + 1], bf16, tag="vaug")
        nc.vector.memset(v_aug[:, :, DK : DK + 1], 1.0)

        # ---- projections ----
        kT_sb = sbufs.tile([128, T], bf16, tag="kT")       # (k^T + bk) * SCALE
        qT_sb = sbufs.tile([128, NSLOT * 128], bf16, tag="qT")  # q^T + bq
        vT_sb = sbufs.tile([128, T], bf16, tag="vT")       # v^T + bv

        # kT gets 4 psum banks, qT 2, vT 2 (accumulated in two half-passes) --
        # all three coexist, so no projection matmul ever waits on another
        # projection's psum release.
        kpool = tc.alloc_tile_pool(name="kpool", bufs=1, space="PSUM")
        qpool = tc.alloc_tile_pool(name="qpool", bufs=1, space="PSUM")
        vpool = tc.alloc_tile_pool(name="vpool", bufs=2, space="PSUM")
        if True:
            kT_ps = kpool.tile([128, T], f32, tag="kps")
            qT_ps = qpool.tile([128, NSLOT * 128], f32, tag="qps")
            # vT accumulates in four [128,512] generations over 2 psum banks;
            # separate tiles (not one [128,1024]) so each b-generation only
            # waits on its own bank's a-copy, not the whole vTa drain
            vps_a = [
                vpool.tile([128, 512], f32, tag="vps", name=f"vpsa{t}")
                for t in range(2)
            ]
            # per chunk: kT x4, vT(first half) x2, qT x2 = 8 matmuls, which
            # matches the x^T chunk DMA arrival rate. Column order t=0,1 first
            # (k then v), then t=2,3: the first half of a chunk's columns
            # arrives first for the split early-chunk DMAs.
            for c in range(NCHUNK):
                for t in range(2):
                    nc.tensor.matmul(
                        kT_ps[:, 512 * t : 512 * (t + 1)],
                        lhsT=wk_sb[:, 128 * c : 128 * (c + 1)],
                        rhs=xts[c][:, 512 * t : 512 * (t + 1)],
                        start=(c == 0),
                        stop=(c == NCHUNK - 1),
                    )
                    nc.tensor.matmul(
                        vps_a[t],
                        lhsT=wv_sb[:, 128 * c : 128 * (c + 1)],
                        rhs=xts[c][:, 512 * t : 512 * (t + 1)],
                        start=(c == 0),
                        stop=(c == NCHUNK - 1),
                    )
                for t in range(2, 4):
                    nc.tensor.matmul(
                        kT_ps[:, 512 * t : 512 * (t + 1)],
                        lhsT=wk_sb[:, 128 * c : 128 * (c + 1)],
                        rhs=xts[c][:, 512 * t : 512 * (t + 1)],
                        start=(c == 0),
                        stop=(c == NCHUNK - 1),
                    )
                x4 = xts[c].rearrange("p (b two x) -> p b two x", two=2, x=128)
                for t in range(2):
                    nc.tensor.matmul(
                        qT_ps[:, 512 * t : 512 * (t + 1)],
                        lhsT=wq_sb[:, 128 * c : 128 * (c + 1)],
                        rhs=x4[:, 4 * t : 4 * t + 4, 1, :],
                        start=(c == 0),
                        stop=(c == NCHUNK - 1),
                    )
            # copies: kT+qT on DVE (the ACT engine is kept free so the first
            # exp can issue the moment S^T_0 lands), vT halves on DVE too.
            # 512-col granularity, interleaved so the first S^T matmul (needs
            # kT cols 0:128 + qT cols 0:512) unblocks after just two copies.
            def kT_copy(t):
                sl = slice(512 * t, 512 * (t + 1))
                nc.vector.tensor_scalar(
                    out=kT_sb[:, sl], in0=kT_ps[:, sl],
                    scalar1=float(SCALE), scalar2=bks_sb,
                    op0=mybir.AluOpType.mult, op1=mybir.AluOpType.add,
                )

            def qT_copy(t):
                sl = slice(512 * t, 512 * (t + 1))
                nc.vector.tensor_scalar_add(qT_sb[:, sl], qT_ps[:, sl], bq_sb)

            kT_copy(0)
            qT_copy(0)
            qT_copy(1)
            kT_copy(1)
            kT_copy(2)
            kT_copy(3)
            for t in range(2):
                sl = slice(512 * t, 512 * (t + 1))
                nc.vector.tensor_scalar_add(vT_sb[:, sl], vps_a[t], bv_sb)

            # vT second half accumulates while the kT/qT copies drain
            vps_b = [
                vpool.tile([128, 512], f32, tag="vps", name=f"vpsb{t}")
                for t in range(2)
            ]
            for c in range(NCHUNK):
                for t in range(2):
                    nc.tensor.matmul(
                        vps_b[t],
                        lhsT=wv_sb[:, 128 * c : 128 * (c + 1)],
                        rhs=xts[c][:, 1024 + 512 * t : 1024 + 512 * (t + 1)],
                        start=(c == 0),
                        stop=(c == NCHUNK - 1),
                    )
            for t in range(2):
                nc.vector.tensor_scalar_add(
                    vT_sb[:, 1024 + 512 * t : 1024 + 512 * (t + 1)],
                    vps_b[t], bv_sb,
                )

        # ---- attention ----
        vpool.release()
        qpool.release()
        kpool.release()
        spool = ctx.enter_context(tc.tile_pool(name="spool", bufs=4, space="PSUM"))
        # one shared 5-slot pool for transpose scratch AND output accumulators:
        # transposes need slots early in the attention phase, o_ps late, so a
        # shared pool gives each phase more slack than a static 1/4 split
        opool = ctx.enter_context(tc.tile_pool(name="opool", bufs=4, space="PSUM"))
        pt_pool = ctx.enter_context(tc.tile_pool(name="pt_pool", bufs=NBLK))

        # v^T -> v natural (bf16) via PE transpose; emitted lazily inside the
        # attention loop so the PE never stalls in a transpose block waiting
        # for the vT copies (transpose for key block j lands just before its
        # S^T matmul; burst p only needs transposes <= 2p+1, which are done)
        def emit_transpose(j):
            vt_ps = opool.tile([128, DK + 1], bf16, tag="o", name=f"vt_ps{j}")
            vt_ps = vt_ps[:, 0:128]
            nc.tensor.transpose(vt_ps, vT_sb[:, 128 * j : 128 * (j + 1)], ident)
            nc.vector.tensor_copy(v_aug[:, j, 0:DK], vt_ps)

        def chunk_sizes(n):
            # pieces <=512, all >=256 when possible (fp32r full-rate needs >=256)
            out = []
            while n > 768:
                out.append(512)
                n -= 512
            if n > 512:
                out.append(n - 256)
                n = 256
            out.append(n)
            return out

        pts = [None] * NBLK

        def pv_mm(o_ps, p, jj):
            nc.tensor.matmul(
                o_ps,
                lhsT=pts[jj][:, 128 * (p - jj // 2) : 128 * (p - jj // 2) + 128],
                rhs=v_aug[:, jj, :],
                start=(jj == 0),
                stop=(jj == 2 * p + 1),
            )

        pair_tiles = {}

        def finish_slot(o_ps, p):
            # copy numerator+denominator (bf16) into the pair staging tile;
            # odd slots copy on ACT so the last two finishes overlap engines.
            # The divide happens on the host.
            g = p // 2
            if g not in pair_tiles:
                pair_tiles[g] = out_pool.tile(
                    [128, 2, DK + 1], bf16, tag="ob", name=f"ob{g}"
                )
            ot = pair_tiles[g]
            if p % 2:
                nc.scalar.copy(ot[:, 1, :], o_ps)
            else:
                nc.vector.tensor_copy(ot[:, 0, :], o_ps)
            if p % 2:
                nc.sync.dma_start(out=o[g], in_=ot)

        # process key positions 14,15 early so the final P@V bursts never
        # wait on their exp at the very end of the kernel
        ORDER = [0, 1, 2, 3, 4, 5, 6, 7, 8, 9, 14, 15, 10, 11, 12, 13]
        done = set()
        burst_done = set()
        half_done = {}          # p -> o_ps, for slots drained progressively
        pending = []            # bursts delayed one ORDER step: their pt dep
                                # (exp) finishes while the next j's S^T matmuls
                                # keep the PE busy, hiding the ACT latency
        SPLIT = {6, 7}          # late slots: drain in three phases so only
                                # blocks 10..13 remain at the kernel tail

        def pv2(o_ps, p, jj, start, stop):
            nc.tensor.matmul(
                o_ps,
                lhsT=pts[jj][:, 128 * (p - jj // 2) : 128 * (p - jj // 2) + 128],
                rhs=v_aug[:, jj, :],
                start=start,
                stop=stop,
            )

        def emit_full(p):
            o_ps = opool.tile([128, DK + 1], f32, tag="o", name=f"o_ps{p}")
            for jj in range(2 * p + 2):
                pv_mm(o_ps, p, jj)
            finish_slot(o_ps, p)

        def emit_phaseA(p):
            o_ps = opool.tile([128, DK + 1], f32, tag="o", name=f"o_ps{p}")
            half_done[p] = o_ps
            for jj in range(8):
                pv2(o_ps, p, jj, start=(jj == 0), stop=False)

        def emit_phaseB(p):
            o_ps = half_done[p]
            for jj in ([8, 9] if p == 6 else [8, 9, 14, 15]):
                pv2(o_ps, p, jj, start=False, stop=False)

        def emit_phaseC(p):
            o_ps = half_done[p]
            for jj in range(10, 14):
                pv2(o_ps, p, jj, start=False, stop=(jj == 13))
            finish_slot(o_ps, p)

        for j in ORDER:
            sj = j // 2           # first active slot for this key position
            q0 = 128 * sj
            qn = NSLOT * 128 - q0
            pt = pt_pool.tile([128, qn], bf16, tag="pt", name=f"pt{j}")
            pts[j] = pt
            off = 0
            for sz in chunk_sizes(qn):
                s_ps = spool.tile([128, 512], f32, tag="st")
                nc.tensor.matmul(
                    s_ps[:, :sz],
                    lhsT=kT_sb[:, 128 * j : 128 * (j + 1)],
                    rhs=qT_sb[:, q0 + off : q0 + off + sz],
                    start=True,
                    stop=True,
                )
                nc.scalar.activation(
                    out=pt[:, off : off + sz], in_=s_ps[:, :sz], func=Act.Exp,
                    scale=1.0,
                )
                if off == 0:
                    # mask the frontier slot multiplicatively (exp(s+m) =
                    # exp(s)*m01): even j -> maskA (wedge-dependent), odd j ->
                    # maskB (causal triangle); bf16 SBUF op, off the psum path
                    sel = j % 2
                    nc.vector.tensor_mul(
                        pt[:, 0:128],
                        pt[:, 0:128],
                        mask_sb[:, 128 * sel : 128 * (sel + 1)],
                    )
                off += sz

            # flush bursts queued on the previous ORDER step, now that this
            # j's score matmuls are in the PE stream ahead of them
            for fn in pending:
                fn()
            pending = []

            emit_transpose(j)
            done.add(j)
            phaseA_done = len(done) >= 8 and all(jj in done for jj in range(8))
            phaseB_done = all(jj in done for jj in [8, 9, 14, 15])
            for p in range(NSLOT):
                if p in burst_done:
                    continue
                if p in SPLIT:
                    if p not in half_done and phaseA_done:
                        pending.append(lambda p=p: emit_phaseA(p))
                        half_done[p] = None  # reserved; tile set in emit_phaseA
                    elif p in half_done and (p, "B") not in burst_done and \
                            phaseA_done and phaseB_done:
                        burst_done.add((p, "B"))
                        pending.append(lambda p=p: emit_phaseB(p))
                    if (p, "B") in burst_done and all(
                        jj in done for jj in range(2 * p + 2)
                    ):
                        burst_done.add(p)
                        pending.append(lambda p=p: emit_phaseC(p))
                elif all(jj in done for jj in range(2 * p + 2)):
                    burst_done.add(p)
                    pending.append(lambda p=p: emit_full(p))
        for fn in pending:
            fn()

    nc.compile()
    return nc


def get_built():
    global _built
    if _built is None:
        _built = _build()
    return _built


def _pos2glob(h):
    if h == 0:
        return list(range(NBLK))
    return [j + 1 if j % 2 == 0 else j - 1 for j in range(NBLK)]


def _xdt():
    if X_BF16:
        import ml_dtypes
        return ml_dtypes.bfloat16
    return np.float32


def _pack_w(W):
    """[D, DK] -> [128, NCHUNK*DK] with column block c holding rows 128c..128c+127."""
    return np.ascontiguousarray(
        np.asarray(W, np.float32).reshape(NCHUNK, 128, DK).transpose(1, 0, 2)
        .reshape(128, NCHUNK * DK).astype(_xdt())
    )


def make_in_map(x_b, Wq, bq, Wk, bk, Wv, bv, h, xT_pre=None):
    """Build one core's input dict. x_b: [T, D] fp32 for this core's batch.
    xT_pre: optional precomputed x_b.T already in the kernel dtype (shared by
    both wedge cores of a batch; h=0 uses it as-is, h=1 column-permutes)."""
    if xT_pre is None:
        xT_pre = np.ascontiguousarray(x_b.T.astype(_xdt()))
    if h == 0:
        xT_loc = xT_pre  # identity key order
    else:
        p2g = _pos2glob(h)
        cols = np.concatenate([np.arange(128 * g, 128 * (g + 1)) for g in p2g])
        xT_loc = np.ascontiguousarray(xT_pre[:, cols])
    import ml_dtypes
    bf = ml_dtypes.bfloat16
    maskA = (np.ones if h == 0 else np.zeros)((128, 128), bf)
    kk = np.arange(128)
    maskB = np.where(kk[:, None] <= kk[None, :], 1.0, 0.0).astype(bf)
    return {
        "xT": xT_loc,
        "wq": _pack_w(Wq),
        "wk": _pack_w(Wk),
        "wv": _pack_w(Wv),
        "bq": np.ascontiguousarray(bq.reshape(DK, 1), np.float32),
        "bks": np.ascontiguousarray((bk * SCALE).reshape(DK, 1), np.float32),
        "bv": np.ascontiguousarray(bv.reshape(DK, 1), np.float32),
        "masks": np.ascontiguousarray(np.concatenate([maskA, maskB], axis=1)),
    }


def gather_out(results):
    """results: list of 8 dicts with 'o' [4, 128, 2, DK+1] bf16 (numerator |
    denominator, slot-paired) -> full [B, T, DK] f32 (softmax divide here)."""
    out = np.zeros((B, T, DK), np.float32)
    for core in range(8):
        b, h = core // 2, core % 2
        ob = np.asarray(results[core]["o"], np.float32)
        for p in range(NSLOT):
            g = 2 * p + 1 - h
            blk = ob[p // 2, :, p % 2, :]
            out[b, 128 * g : 128 * (g + 1), :] = blk[:, :DK] / blk[:, DK:]
    return out


def kernel(x, Wq, bq, Wk, bk, Wv, bv):
    from concourse.bass_utils import run_bass_kernel_spmd

    x = np.asarray(x, np.float32)
    args = [np.asarray(a, np.float32) for a in (Wq, bq, Wk, bk, Wv, bv)]
    nc = get_built()
    # one transpose+cast per batch, shared by its two wedge cores
    xT_pres = [np.ascontiguousarray(x[b].T.astype(_xdt())) for b in range(B)]
    in_maps = [
        make_in_map(x[core // 2], args[0], args[1], args[2], args[3], args[4],
                    args[5], core % 2, xT_pre=xT_pres[core // 2])
        for core in range(8)
    ]
    res = run_bass_kernel_spmd(nc, in_maps, core_ids=list(range(8)))
    return gather_out(res.results)


if __name__ == "__main__":
    rng = np.random.default_rng(0)
    x = rng.standard_normal((B, T, D), dtype=np.float32)
    Wq = rng.standard_normal((D, DK), dtype=np.float32) * 0.03
    out = kernel(x, Wq, np.zeros(DK, np.float32), Wq, np.zeros(DK, np.float32),
                 Wq, np.zeros(DK, np.float32))
    print(out.shape)

